# revision 2
# baseline (speedup 1.0000x reference)
"""Depthwise 4x4 blur (upfirdn2d pad=(2,1)) on TRN2, 8 NeuronCores — v4.2.

Same math as v3 (banded-matrix matmuls over H, separable-pair DVE prep over W,
bf16 I/O); schedule rebuilt around measured DMA/engine facts:

  - Per-core DMA: ~342 GB/s on one HWDGE queue, ~405-424 GB/s with both.
    Whole input+output fits in SBUF, so ALL input DMAs are issued up front.
  - HWDGE queues block the ISSUING engine's sequencer when >~8 DMAs are
    outstanding, so the ACT engine (which must run PSUM->SBUF copies) gets
    NO input triggers: input rides the sync queue (SP blocks harmlessly),
    output+weights ride the scalar queue, production-paced (never >2 deep).
  - Queue BYTES must balance: qSP = 8.7MB input, qAct = 8.5MB output+wts.
  - Per-chunk DMA cost = max(bytes/rate, 128 descriptors x ~160ns / 16
    engines): descriptors are kept at 4KB via max_dma_last_dim (8KB
    descriptors trigger a ~25% slowdown on DMA engine 15).
  - DVE tensor_tensor runs 0.60 ns/elem on large contiguous ops vs 0.66
    strided (no 2x bf16 mode): preps are emitted as per-input-chunk "mega"
    ops computing U = x[m]+s_u*x[m+3], V = x[m+1]+s_v*x[m+2] over the whole
    chunk; the image-seam garbage cols are never read by the matmul views.
  - Pipeline spine: DVE megas (~157 ns/image) pace the kernel; PE/DVE are
    balanced by running 2.5 of the 11 input chunks on the raw 4-matmul path
    (head chunk so PE starts before any prep exists, one mid chunk).
  - PSUM: 2 quad tiles (4 banks each), one 2048-col ACT copy per quad; the
    last quad uses per-group copies alternating ACT/DVE, emitted BEFORE the
    tail output triggers so a blocked trigger can't delay them.
"""

import numpy as np
from contextlib import ExitStack

import concourse.bass as bass
import concourse.bacc as bacc
import concourse.tile as tile
import concourse.mybir as mybir
from concourse.bass_utils import run_bass_kernel_spmd

N_CORES = 8
B, C, H, W = 8, 256, 128, 128
WP = W + 3         # padded image stride: [0, 0, x0..x127, 0]
GROUP = 4          # images per matmul group (512 f32 = one PSUM bank)
QUAD = 4           # groups per PSUM tile / ACT copy

F32 = mybir.dt.float32
BF16 = mybir.dt.bfloat16
MULT = mybir.AluOpType.mult
ADD = mybir.AluOpType.add

# v4.2 lesson: input on one queue + output on the other = input crawls at the
# ~211 GB/s fair share (done ~49.7us) and the whole pipeline slides. Instead:
# input is split across BOTH queues upfront (queues run input first at
# ~415 GB/s, done ~30us), output descriptors queue up behind it and drain at
# ~417 GB/s until ~50us. ACT's upfront trigger count stays at 7 (6 input + 1
# wts), under the ~8-outstanding HWDGE cap that blocks the sequencer.
# Input chunks alternate sync/scalar starting with sync (c0 lands ~9.7us).
IN_SIZES = [8, 8, 16, 16, 16, 16, 16, 32, 32, 32, 32, 16, 16]
# v4.3 lesson: chunk-level path grouping (mega-preps) makes PE bursty — every
# resume pays a visible LDWEIGHTS (+160ns) and chunk-long DVE runs leave PE
# idle 0.6us/chunk. Per-group strided preps interleaved with a 4-matmul group
# every 6th keep both engines locally balanced (v4.1's PE finished 3us sooner
# than v4.3's despite more matmuls).
MM4_EVERY = 6
# output chunks alternate scalar/sync (scalar first), small tail
OUT_SIZES = [32, 32, 32, 32, 32, 32, 32, 16, 8, 8]
assert sum(IN_SIZES) == C and sum(OUT_SIZES) == C


def _body(ctx, tc, o_ap, x_ap, w_ap, s_u, s_v):
    nc = tc.nc
    wpool = ctx.enter_context(tc.tile_pool(name="wts", bufs=1))
    xpool = ctx.enter_context(tc.tile_pool(name="xin", bufs=1))
    upool = ctx.enter_context(tc.tile_pool(name="uv", bufs=12))
    opool = ctx.enter_context(tc.tile_pool(name="oup", bufs=1))
    ppool = ctx.enter_context(tc.tile_pool(name="ps", bufs=2, space="PSUM"))

    wt = wpool.tile([H, 4 * H], BF16)
    xt = xpool.tile([H, C * WP], BF16)
    xt3 = xt[:].rearrange("h (c w) -> h c w", c=C)
    ot = opool.tile([H, C * W], BF16)

    # weights FIRST on the scalar queue (land ~9.8us, gate only the first
    # matmul); input chunks upfront alternating sync/scalar.
    nc.scalar.dma_start(wt[:], w_ap)
    in_off = [0]
    for i, sz in enumerate(IN_SIZES):
        o = in_off[-1]
        eng = nc.sync if i % 2 == 0 else nc.scalar
        eng.dma_start(
            xt3[:, o : o + sz], x_ap[:, o : o + sz],
            max_dma_last_dim=16 * WP,
        )
        in_off.append(o + sz)

    # PE warmup: HAM clock gate needs ~3.4us of sustained activity for
    # 2.4 GHz; dummy matmuls on a GpSimd-memset scratch run inside the
    # framework preamble shadow.
    scratch = wpool.tile([H, 4 * W], BF16)
    nc.gpsimd.memset(scratch[:], 0)

    n_groups = C // GROUP           # 64
    n_quads = n_groups // QUAD      # 16

    def path_of(g):
        # head groups mm4 (PE starts before any prep exists), tail groups mm4
        # (PE closes), every MM4_EVERYth in between for PE/DVE balance
        if g < 2 or g >= n_groups - 2:
            return "mm4"
        return "mm4" if (g - 1) % MM4_EVERY == 0 else "dve"

    def emit_prep(uv3, gi, gc):
        for k, (ja, jb, s) in enumerate(((0, 3, s_u), (1, 2, s_v))):
            va = xt3[:, gi : gi + gc, ja : ja + W]
            vb = xt3[:, gi : gi + gc, jb : jb + W]
            if s == 1.0:
                nc.vector.tensor_tensor(uv3[:, k], va, vb, ADD)
            else:
                nc.vector.scalar_tensor_tensor(uv3[:, k], vb, s, va, MULT, ADD)

    pt0 = ppool.tile([H, QUAD * GROUP * W], F32, tag="pt")
    for _ in range(9):
        nc.tensor.matmul(pt0[:, :512], scratch[:, :H], scratch[:], start=True, stop=True)

    out_bounds = []
    acc = 0
    for sz in OUT_SIZES:
        acc += sz
        out_bounds.append(acc)

    out_idx = 0
    out_off = 0

    def emit_out_triggers(copied):
        nonlocal out_idx, out_off
        while out_idx < len(OUT_SIZES) and out_bounds[out_idx] <= copied:
            sz = OUT_SIZES[out_idx]
            eng = nc.scalar if out_idx % 2 == 0 else nc.sync
            eng.dma_start(
                o_ap[:, out_off : out_off + sz],
                ot[:, out_off * W : (out_off + sz) * W].rearrange(
                    "h (c w) -> h c w", c=sz
                ),
                max_dma_last_dim=16 * W,
            )
            out_off += sz
            out_idx += 1

    for q in range(n_quads):
        if q == 0:
            pts = [pt0]
        elif q < n_quads - 1:
            pts = [ppool.tile([H, QUAD * GROUP * W], F32, tag="pt", name=f"pt{q}")]
        else:
            # last quad: TWO 2-bank tiles so the two closing pair-copies read
            # different PSUM tiles — copies reading the SAME tile serialize
            # even across engines (measured: DVE CAST starts 33ns after the
            # ACT copy ends, every variant)
            pts = [
                ppool.tile([H, 2 * GROUP * W], F32, tag="pt", name="ptA"),
                ppool.tile([H, 2 * GROUP * W], F32, tag="pt", name="ptB"),
            ]
        for j in range(QUAD):
            g = q * QUAD + j
            gi = g * GROUP
            if len(pts) == 1:
                ps = pts[0][:, j * GROUP * W : (j + 1) * GROUP * W]
            else:
                ps = pts[j // 2][:, (j % 2) * GROUP * W : (j % 2 + 1) * GROUP * W]
            if path_of(g) == "dve":
                uv = upool.tile([H, 2 * GROUP * W], BF16, tag="uv")
                uv3 = uv[:].rearrange("h (k c w) -> h k c w", k=2, c=GROUP)
                emit_prep(uv3, gi, GROUP)
                nc.tensor.matmul(ps, wt[:, :H], uv3[:, 0], start=True, stop=False)
                nc.tensor.matmul(ps, wt[:, H : 2 * H], uv3[:, 1],
                                 start=False, stop=True)
            else:
                for t in range(4):
                    nc.tensor.matmul(
                        ps, wt[:, t * H : (t + 1) * H],
                        xt3[:, gi : gi + GROUP, t : t + W],
                        start=(t == 0), stop=(t == 3),
                    )
        qi = q * QUAD * GROUP  # first image of quad
        if q < n_quads - 1:
            nc.scalar.copy(ot[:, qi * W : (qi + QUAD * GROUP) * W], pts[0][:])
            emit_out_triggers(qi + QUAD * GROUP)
        else:
            # last quad: two PAIR copies in parallel on ACT and DVE, reading
            # the two separate PSUM tiles. Emitted before the tail triggers
            # so a waiting trigger can't delay them.
            half = 2 * GROUP * W
            nc.scalar.copy(ot[:, qi * W : qi * W + half], pts[0][:])
            nc.vector.tensor_copy(
                ot[:, qi * W + half : qi * W + 2 * half], pts[1][:]
            )
            emit_out_triggers(C)


def build_module(s_u, s_v):
    nc = bacc.Bacc(
        "TRN2", target_bir_lowering=False, debug=False, num_devices=N_CORES
    )
    x_ap = nc.dram_tensor("x", [H, C, WP], BF16, kind="ExternalInput").ap()
    w_ap = nc.dram_tensor("wts", [H, 4 * H], BF16, kind="ExternalInput").ap()
    o_ap = nc.dram_tensor("out", [H, C, W], BF16, kind="ExternalOutput").ap()
    with tile.TileContext(nc) as tc:
        with ExitStack() as ctx:
            _body(ctx, tc, o_ap, x_ap, w_ap, s_u, s_v)
    nc.compile()
    return nc


def band_mats(k2d):
    """WT[j] = A_j^T where A_j[h, h+i-2] = Kf[i, j] (rows clipped to [0,128))."""
    kf = np.asarray(k2d, np.float32)[::-1, ::-1]
    wts = np.zeros((4, H, H), np.float32)
    for j in range(4):
        for i in range(4):
            d = i - 2  # diagonal offset m - h
            h0, h1 = max(0, -d), min(H, H - d)
            idx = np.arange(h0, h1)
            wts[j, idx + d, idx] = kf[i, j]
    return wts


def _bf16(a):
    import ml_dtypes

    return np.asarray(a).astype(ml_dtypes.bfloat16)


def prep_x(x_core):
    """[C,H,W] f32 -> [H,C,WP] bf16 with zero cols at 0,1 and WP-1."""
    xp = np.zeros((H, x_core.shape[0], WP), np.float32)
    xp[:, :, 2 : 2 + W] = x_core.transpose(1, 0, 2)
    return _bf16(xp)


_module_cache = {}


def kernel(x, kernel, _trace=False, _trace_kwargs=None):
    x = np.asarray(x, np.float32)
    assert x.shape == (B, C, H, W), x.shape
    kf = np.asarray(kernel, np.float32)[::-1, ::-1]
    kw = kf.sum(axis=0) / kf.sum()
    s_u = float(kw[3] / kw[0])  # u = Xp0 + s_u*Xp3 under stationary A_0
    s_v = float(kw[2] / kw[1])  # v = Xp1 + s_v*Xp2 under stationary A_1
    key = (round(s_u, 9), round(s_v, 9))
    if key not in _module_cache:
        _module_cache[key] = build_module(s_u, s_v)
    nc = _module_cache[key]
    wts = _bf16(band_mats(kernel).transpose(1, 0, 2).reshape(H, 4 * H))
    in_maps = [{"x": prep_x(x[i]), "wts": wts.copy()} for i in range(N_CORES)]
    res = run_bass_kernel_spmd(
        nc, in_maps, list(range(N_CORES)), trace=_trace, **(_trace_kwargs or {})
    )
    out = np.stack(
        [
            np.asarray(res.results[i]["out"]).transpose(1, 0, 2).astype(np.float32)
            for i in range(N_CORES)
        ],
        axis=0,
    )
    if _trace:
        return out, res
    return out
